# revision 31
# baseline (speedup 1.0000x reference)
"""Expert-parallel MoE FFN kernel for Trainium2 (8 NeuronCores, one expert per core).

Host side: routes tokens to experts (dedup per expert, summing duplicate top-k
weights), pads each expert's token list to the max expert count t_pad (NOT
rounded to 512 — matmul free dim is arbitrary <=512), and pre-tiles the weight
matrices into DMA-friendly contiguous blocks.

Device side (per core, expert e):
  h^T = silu(G_e^T X^T) * (U_e^T X^T)        [I, T]   (stage A)
  y^T = (D^T h^T) * cw                        [H, T]   (stage B)
All matmuls in bf16 (1 col/cycle on the PE), fp32 accumulation in PSUM.

Perf-critical structure (from NTFF trace analysis of previous versions):
 - The token dim is split in two EQUAL chunks (csz = t_pad/2 <= 512); the two
   PSUM accumulators for a (gate|up|out) pair live in ONE [P, 2, 512] tile
   spanning two adjacent banks, so silu / h-mul / cw-mul / y-DMA each cover
   both chunks with a single instruction. Fewer instructions -> fewer
   cross-engine event semaphores -> a much shorter end-of-kernel semaphore
   reset storm (which counts toward measured exec time).
 - ALL input DMAs stream on the sync queue in strict deadline order
   (X k=0, G0, U0, then X k-chunks, then batched weights): one queue at
   the full ~358GB/s per-core rate beats parallel queues at 1/3 rate
   each, because the startup tiles are consumed sequentially.
 - ~38 warmup matmuls on a memset tile run during the initial DMA wait so the
   PE HAM clock-gate reaches 2.4 GHz before real matmuls start (plus a few
   interleaved in the first i-iteration to bridge X-arrival jitter).
 - PSUM is hand-scheduled as 4 explicit pair tiles. Stage A double-buffers
   (even i -> pb0/pb1, odd i -> pb2/pb3); stage B's first accumulator group
   reuses pb0/pb1 (freed mid stage A) so the tensor engine never idles at
   the A->B transition. Stage B's last two groups are single-jj so the
   post-last-matmul tail is one mul + one DMA.
"""
import sys

if "/opt/trn_rl_repo" not in sys.path:
    sys.path.insert(0, "/opt/trn_rl_repo")

import numpy as np

N_TOKENS, TOP_K, N_EXPERTS, HIDDEN, INTER = 4096, 2, 8, 1024, 2048
P = 128
NI = INTER // P          # 16 I-tiles
KH = HIDDEN // P         # 8 H(contraction)-tiles
N_WARMUP = 24

_CACHE = {}


_SEM_PATCHED = False


def _shrink_sem_pool(cap=128):
    """Cap the kernel event-semaphore pool. The NEFF epilogue individually
    resets every semaphore in the pool (~25ns each across the engines); the
    kernel only uses ~20, so the default 254-wide pool costs ~6us of measured
    tail. A smaller pool shrinks that reset storm proportionally."""
    global _SEM_PATCHED
    if _SEM_PATCHED:
        return
    import concourse.bass as bass_mod

    orig = bass_mod.get_kernel_semaphore_range

    def small_range():
        r = orig()
        return range(r.start, min(r.stop, r.start + cap))

    bass_mod.get_kernel_semaphore_range = small_range
    _SEM_PATCHED = True


def _build(t_pad):
    import concourse.bacc as bacc
    import concourse.mybir as mybir
    import concourse.tile as tile

    _shrink_sem_pool()

    f32 = mybir.dt.float32
    bf16 = mybir.dt.bfloat16
    tp = t_pad
    assert tp % 8 == 0 and tp <= 1024, f"t_pad {tp} unsupported"
    ntc = 1 if tp <= 512 else 2
    csz = tp // ntc

    nc = bacc.Bacc()
    xt = nc.declare_dram_parameter("xt", [P, KH * tp], bf16, isOutput=False)
    gw = nc.declare_dram_parameter("gw", [P, NI * HIDDEN], bf16, isOutput=False)
    uw = nc.declare_dram_parameter("uw", [P, NI * HIDDEN], bf16, isOutput=False)
    dw = nc.declare_dram_parameter("dw", [NI // 4, P, 4 * HIDDEN], bf16,
                                   isOutput=False)
    cw = nc.declare_dram_parameter("cw", [P, tp], f32, isOutput=False)
    y = nc.declare_dram_parameter("y", [HIDDEN, tp], bf16, isOutput=True)

    H = HIDDEN

    with tile.TileContext(nc) as tc:
        with (
            tc.tile_pool(name="ps", bufs=1, space="PSUM") as ps,
            tc.tile_pool(name="sb", bufs=1) as sb,
            tc.tile_pool(name="sm", bufs=2) as sm,
        ):
            # --- startup-critical DMAs, one per DMA-capable queue ---
            wsrc = sb.tile([P, P], bf16, tag="wsrc", name="wsrc")
            nc.gpsimd.memset(wsrc[:], 0.0)

            # X chunked in k-consumption order. Tiles keep >=2KB per-partition
            # rows — sub-2KB descriptor rows halve early DMA throughput,
            # which is what actually gates the startup.
            xkmap = {}
            xtiles = []
            for ci, ks in enumerate(((0,), (1,), (2, 3), (4, 5), (6, 7))):
                t = sb.tile([P, len(ks) * tp], bf16, tag=f"x{ci}", name=f"x{ci}")
                xtiles.append((t, ks[0], len(ks)))
                for k in ks:
                    xkmap[k] = (t, k - ks[0])
            nc.sync.dma_start(out=xtiles[0][0][:], in_=xt[:, 0:tp])

            def xk(k, c):
                t, off = xkmap[k]
                return t[:, off * tp + c * csz: off * tp + (c + 1) * csz]

            # weight chunk tiles: [start_i, n_i) each; g0/u0 single tiles on
            # their own queues so the first iteration's weights arrive early
            WCH = ((0, 1), (1, 1), (2, 2), (4, 4), (8, 4), (12, 4))
            gts = {}
            uts = {}
            gts[WCH[0]] = sb.tile([P, H], bf16, tag="g0", name="g0")
            nc.sync.dma_start(out=gts[WCH[0]][:], in_=gw[:, 0:H])
            uts[WCH[0]] = sb.tile([P, H], bf16, tag="u0", name="u0")
            nc.sync.dma_start(out=uts[WCH[0]][:], in_=uw[:, 0:H])

            def wslice(tiles, i, k):
                for (i0, n), t in tiles.items():
                    if i0 <= i < i0 + n:
                        return t[:, (i - i0) * H + k * P:
                                 (i - i0) * H + (k + 1) * P]
                raise KeyError(i)

            # remaining X chunks next on sync, then weights by deadline
            for t, k0, nk in xtiles[1:]:
                nc.sync.dma_start(out=t[:], in_=xt[:, k0 * tp:(k0 + nk) * tp])
            for i0, n in WCH[1:]:
                gts[(i0, n)] = sb.tile([P, n * H], bf16, tag=f"g{i0}",
                                       name=f"g{i0}")
                nc.sync.dma_start(out=gts[(i0, n)][:],
                                  in_=gw[:, i0 * H:(i0 + n) * H])
                uts[(i0, n)] = sb.tile([P, n * H], bf16, tag=f"u{i0}",
                                       name=f"u{i0}")
                nc.sync.dma_start(out=uts[(i0, n)][:],
                                  in_=uw[:, i0 * H:(i0 + n) * H])

            dts = []
            for q in range(4):
                t = sb.tile([P, 4 * H], bf16, tag=f"dw{q}", name=f"dwt{q}")
                nc.sync.dma_start(out=t[:], in_=dw[q])
                dts.append(t)

            def dslice(i, jj):
                q, r = divmod(i, 4)
                return dts[q][:, r * H + jj * P: r * H + (jj + 1) * P]

            cwt = sb.tile([P, ntc, csz], f32, tag="cw", name="cwt")
            nc.sync.dma_start(out=cwt[:], in_=cw[:])

            def pair(idx, name):
                return ps.tile([P, ntc, 512], f32, tag=f"pb{idx}", name=name)

            # --- PE warmup during the input DMA wait ---
            wps = pair(3, "warm_ps")
            for r in range(N_WARMUP):
                nc.tensor.matmul(out=wps[:, ntc - 1, 0:P], lhsT=wsrc[:],
                                 rhs=wsrc[:], start=True, stop=True)

            hts = [sb.tile([P, ntc, csz], bf16, tag=f"h{i}", name=f"ht{i}")
                   for i in range(NI)]

            # ---- Stage A ----
            # chunk c=0 consumed before c=1 per k so the first matmul's DMA
            # dependency is a single half-tile; a few warmup MMs after the
            # first k-groups bridge DMA-arrival jitter without idling the PE
            for i in range(NI):
                pg = pair(2 * (i % 2), f"pg{i}")
                pu = pair(2 * (i % 2) + 1, f"pu{i}")
                for k in range(KH):
                    lg = wslice(gts, i, k)
                    lu = wslice(uts, i, k)
                    for c in range(ntc):
                        nc.tensor.matmul(out=pg[:, c, 0:csz], lhsT=lg,
                                         rhs=xk(k, c),
                                         start=(k == 0), stop=(k == KH - 1))
                    for c in range(ntc):
                        nc.tensor.matmul(out=pu[:, c, 0:csz], lhsT=lu,
                                         rhs=xk(k, c),
                                         start=(k == 0), stop=(k == KH - 1))
                    if i == 0 and k < 2:
                        # bridge X-arrival jitter so HAM never re-throttles
                        for r in range(4):
                            nc.tensor.matmul(out=wps[:, ntc - 1, 0:P],
                                             lhsT=wsrc[:], rhs=wsrc[:],
                                             start=True, stop=True)
                sg = sm.tile([P, ntc, csz], f32, tag="sg", name=f"sg{i}")
                nc.scalar.activation(out=sg[:], in_=pg[:, :, 0:csz],
                                     func=mybir.ActivationFunctionType.Silu)
                nc.vector.tensor_mul(out=hts[i][:], in0=sg[:],
                                     in1=pu[:, :, 0:csz])

            # ---- Stage B ----
            # groups of <=2 jj (<=4 PSUM banks); group 0 overlaps the stage A
            # tail. The final single-jj group's readout is split per chunk
            # with the two DMAs on different queues for the shortest tail.
            jgroups = [(0, 1), (2, 3), (4, 5), (6,), (7,)]
            for g, grp in enumerate(jgroups):
                base = 2 * (g % 2)
                if len(grp) > 1 or ntc == 1:
                    pys = {jj: pair(base + a, f"py{jj}")
                           for a, jj in enumerate(grp)}
                    for i in range(NI):
                        for jj in grp:
                            ld = dslice(i, jj)
                            for c in range(ntc):
                                nc.tensor.matmul(out=pys[jj][:, c, 0:csz],
                                                 lhsT=ld,
                                                 rhs=hts[i][:, c, 0:csz],
                                                 start=(i == 0),
                                                 stop=(i == NI - 1))
                    for a, jj in enumerate(grp):
                        yb = sm.tile([P, ntc, csz], bf16, tag=f"yb{jj % 2}",
                                     name=f"yb{jj}")
                        nc.vector.tensor_mul(out=yb[:],
                                             in0=pys[jj][:, :, 0:csz],
                                             in1=cwt[:])
                        eng = nc.gpsimd if jj % 2 else nc.sync
                        eng.dma_start(out=y[jj * P:(jj + 1) * P, :], in_=yb[:])
                    continue
                # single-jj tail group: chunk-major so chunk c's readout and
                # DMA hide under chunk c+1's matmuls. Each chunk gets its OWN
                # PSUM tile — sharing the pair tile would make c1's first
                # matmul wait on c0's readout mul (WAR on the tile). The
                # final chunk's DMA is split across both queues.
                jj = grp[0]
                last_grp = g == len(jgroups) - 1
                accs = [pair(base, f"py{jj}c0"), pair(base + 1, f"py{jj}c1")]
                for c in range(ntc):
                    acc = accs[c][:, c, 0:csz]
                    for i in range(NI):
                        nc.tensor.matmul(out=acc,
                                         lhsT=dslice(i, jj),
                                         rhs=hts[i][:, c, 0:csz],
                                         start=(i == 0), stop=(i == NI - 1))
                    ybc = sm.tile([P, csz], bf16, tag=f"ybc{c}",
                                  name=f"ybc{jj}_{c}")
                    nc.vector.tensor_mul(out=ybc[:],
                                         in0=acc,
                                         in1=cwt[:, c, :])
                    if last_grp and c == ntc - 1:
                        half = csz // 2
                        nc.sync.dma_start(
                            out=y[jj * P:(jj + 1) * P, c * csz:c * csz + half],
                            in_=ybc[:, 0:half])
                        nc.gpsimd.dma_start(
                            out=y[jj * P:(jj + 1) * P,
                                  c * csz + half:(c + 1) * csz],
                            in_=ybc[:, half:csz])
                    else:
                        eng = nc.gpsimd if c % 2 else nc.sync
                        eng.dma_start(
                            out=y[jj * P:(jj + 1) * P, c * csz:(c + 1) * csz],
                            in_=ybc[:])

    nc.finalize()
    return nc


def _route(expert_indices, expert_weights):
    idx = np.asarray(expert_indices).astype(np.int64)
    wts = np.asarray(expert_weights).astype(np.float32)
    n = idx.shape[0]
    cw_full = np.zeros((N_EXPERTS, n), np.float32)
    for k in range(idx.shape[1]):
        np.add.at(cw_full, (idx[:, k], np.arange(n)), wts[:, k])
    ids = [np.nonzero(cw_full[e])[0] for e in range(N_EXPERTS)]
    maxc = max(len(i) for i in ids)
    t_pad = max(512, ((maxc + 7) // 8) * 8)
    return cw_full, ids, t_pad


def _run(nc, in_maps, trace=False, trace_cores=None):
    from concourse.bass_utils import run_bass_kernel_spmd

    return run_bass_kernel_spmd(
        nc, in_maps, list(range(N_EXPERTS)), trace=trace,
        trace_cores=trace_cores,
    )


def _make_in_maps(tokens, gwl, uwl, dwl, cw_full, ids, t_pad):
    bf16 = np.dtype("bfloat16")
    in_maps = []
    for e in range(N_EXPERTS):
        ce = len(ids[e])
        xe = np.zeros((HIDDEN, t_pad), np.float32)
        xe[:, :ce] = tokens[ids[e]].T
        cwe = np.zeros((t_pad,), np.float32)
        cwe[:ce] = cw_full[e, ids[e]]
        in_maps.append({
            # [P, KH*tp]: xt[p, k*tp+t] = X^T[k*128+p, t]
            "xt": np.ascontiguousarray(
                xe.reshape(KH, P, t_pad).transpose(1, 0, 2)
            ).reshape(P, KH * t_pad).astype(bf16),
            # [P, NI*H]: gw[p, i*H + k*128+q] = G[k*128+p, i*128+q]
            "gw": gwl[e],
            "uw": uwl[e],
            # [4, P, 4*H]: dw[q][p][r*H+h] = D[(4q+r)*128+p, h]
            "dw": dwl[e],
            "cw": np.ascontiguousarray(
                np.broadcast_to(cwe[None, :], (P, t_pad))),
        })
    return in_maps


def prepare(tokens, expert_indices, expert_weights, gate_weight, up_weight,
            down_weight):
    """Host-side routing + layout. Returns (nc, in_maps, ids, t_pad)."""
    tokens = np.ascontiguousarray(np.asarray(tokens, dtype=np.float32))
    gate_weight = np.asarray(gate_weight, dtype=np.float32)
    up_weight = np.asarray(up_weight, dtype=np.float32)
    down_weight = np.asarray(down_weight, dtype=np.float32)

    cw_full, ids, t_pad = _route(expert_indices, expert_weights)

    key = t_pad
    if key not in _CACHE:
        _CACHE[key] = _build(t_pad)
    nc = _CACHE[key]

    bf16 = np.dtype("bfloat16")
    gwl, uwl, dwl = [], [], []
    for e in range(N_EXPERTS):
        gwl.append(np.ascontiguousarray(
            gate_weight[e].reshape(KH, P, NI, P).transpose(1, 2, 0, 3)
        ).reshape(P, NI * HIDDEN).astype(bf16))
        uwl.append(np.ascontiguousarray(
            up_weight[e].reshape(KH, P, NI, P).transpose(1, 2, 0, 3)
        ).reshape(P, NI * HIDDEN).astype(bf16))
        dwl.append(np.ascontiguousarray(
            down_weight[e].reshape(4, 4, P, HIDDEN).transpose(0, 2, 1, 3)
        ).reshape(4, P, 4 * HIDDEN).astype(bf16))

    in_maps = _make_in_maps(tokens, gwl, uwl, dwl, cw_full, ids, t_pad)
    return nc, in_maps, ids, t_pad


def combine(results, ids):
    out = np.zeros((N_TOKENS, HIDDEN), np.float32)
    for e in range(N_EXPERTS):
        ce = len(ids[e])
        out[ids[e]] += results[e]["y"].T[:ce].astype(np.float32)
    return out


def kernel(tokens, expert_indices, expert_weights, gate_weight, up_weight,
           down_weight):
    tokens = np.ascontiguousarray(np.asarray(tokens, dtype=np.float32))
    cw_full, ids, t_pad = _route(expert_indices, expert_weights)
    if t_pad <= 1024:
        nc, in_maps, ids, _ = prepare(tokens, expert_indices, expert_weights,
                                      gate_weight, up_weight, down_weight)
        res = _run(nc, in_maps, trace=False)
        return combine(res.results, ids)

    # fallback for extreme routing skew: split each expert's token list into
    # segments of <=1024 and run one SPMD launch per segment
    gate_weight = np.asarray(gate_weight, dtype=np.float32)
    up_weight = np.asarray(up_weight, dtype=np.float32)
    down_weight = np.asarray(down_weight, dtype=np.float32)
    bf16 = np.dtype("bfloat16")
    gwl, uwl, dwl = [], [], []
    for e in range(N_EXPERTS):
        gwl.append(np.ascontiguousarray(
            gate_weight[e].reshape(KH, P, NI, P).transpose(1, 2, 0, 3)
        ).reshape(P, NI * HIDDEN).astype(bf16))
        uwl.append(np.ascontiguousarray(
            up_weight[e].reshape(KH, P, NI, P).transpose(1, 2, 0, 3)
        ).reshape(P, NI * HIDDEN).astype(bf16))
        dwl.append(np.ascontiguousarray(
            down_weight[e].reshape(4, 4, P, HIDDEN).transpose(0, 2, 1, 3)
        ).reshape(4, P, 4 * HIDDEN).astype(bf16))

    nseg = (max(len(i) for i in ids) + 1023) // 1024
    out = np.zeros((N_TOKENS, HIDDEN), np.float32)
    for s in range(nseg):
        sids = [i[s * 1024:(s + 1) * 1024] for i in ids]
        maxc = max((len(i) for i in sids), default=0)
        if maxc == 0:
            continue
        seg_pad = max(512, ((maxc + 7) // 8) * 8)
        if seg_pad not in _CACHE:
            _CACHE[seg_pad] = _build(seg_pad)
        in_maps = _make_in_maps(tokens, gwl, uwl, dwl, cw_full, sids, seg_pad)
        res = _run(_CACHE[seg_pad], in_maps, trace=False)
        out += combine(res.results, sids)
    return out


# revision 33
# speedup vs baseline: 1.0036x; 1.0036x over previous
"""Expert-parallel MoE FFN kernel for Trainium2 (8 NeuronCores, one expert per core).

Host side: routes tokens to experts (dedup per expert, summing duplicate top-k
weights), pads each expert's token list to the max expert count t_pad (NOT
rounded to 512 — matmul free dim is arbitrary <=512), and pre-tiles the weight
matrices into DMA-friendly contiguous blocks.

Device side (per core, expert e):
  h^T = silu(G_e^T X^T) * (U_e^T X^T)        [I, T]   (stage A)
  y^T = (D^T h^T) * cw                        [H, T]   (stage B)
All matmuls in bf16 (1 col/cycle on the PE), fp32 accumulation in PSUM.

Perf-critical structure (from NTFF trace analysis of previous versions):
 - The token dim is split in two EQUAL chunks (csz = t_pad/2 <= 512); the two
   PSUM accumulators for a (gate|up|out) pair live in ONE [P, 2, 512] tile
   spanning two adjacent banks, so silu / h-mul / cw-mul / y-DMA each cover
   both chunks with a single instruction. Fewer instructions -> fewer
   cross-engine event semaphores -> a much shorter end-of-kernel semaphore
   reset storm (which counts toward measured exec time).
 - Input DMAs are spread across the three DMA-capable queues (sync/scalar/
   gpsimd) with the startup-critical tiles (X k=0, G0, U0) first; X is
   chunked 5 ways so arrival tracks the k-consumption order of the first
   i-iteration. Weight DMAs are batched (fewer events) and ordered by
   deadline on the sync queue.
 - ~38 warmup matmuls on a memset tile run during the initial DMA wait so the
   PE HAM clock-gate reaches 2.4 GHz before real matmuls start (plus a few
   interleaved in the first i-iteration to bridge X-arrival jitter).
 - PSUM is hand-scheduled as 4 explicit pair tiles. Stage A double-buffers
   (even i -> pb0/pb1, odd i -> pb2/pb3); stage B's first accumulator group
   reuses pb0/pb1 (freed mid stage A) so the tensor engine never idles at
   the A->B transition. Stage B's last two groups are single-jj so the
   post-last-matmul tail is one mul + one DMA.
"""
import sys

if "/opt/trn_rl_repo" not in sys.path:
    sys.path.insert(0, "/opt/trn_rl_repo")

import numpy as np

N_TOKENS, TOP_K, N_EXPERTS, HIDDEN, INTER = 4096, 2, 8, 1024, 2048
P = 128
NI = INTER // P          # 16 I-tiles
KH = HIDDEN // P         # 8 H(contraction)-tiles
N_WARMUP = 38

_CACHE = {}


_SEM_PATCHED = False


def _shrink_sem_pool(cap=128):
    """Cap the kernel event-semaphore pool. The NEFF epilogue individually
    resets every semaphore in the pool (~25ns each across the engines); the
    kernel only uses ~20, so the default 254-wide pool costs ~6us of measured
    tail. A smaller pool shrinks that reset storm proportionally."""
    global _SEM_PATCHED
    if _SEM_PATCHED:
        return
    import concourse.bass as bass_mod

    orig = bass_mod.get_kernel_semaphore_range

    def small_range():
        r = orig()
        return range(r.start, min(r.stop, r.start + cap))

    bass_mod.get_kernel_semaphore_range = small_range
    _SEM_PATCHED = True


def _build(t_pad):
    import concourse.bacc as bacc
    import concourse.mybir as mybir
    import concourse.tile as tile

    _shrink_sem_pool()

    f32 = mybir.dt.float32
    bf16 = mybir.dt.bfloat16
    tp = t_pad
    assert tp % 8 == 0 and tp <= 1024, f"t_pad {tp} unsupported"
    ntc = 1 if tp <= 512 else 2
    csz = tp // ntc

    nc = bacc.Bacc()
    xt = nc.declare_dram_parameter("xt", [P, KH * tp], bf16, isOutput=False)
    gw = nc.declare_dram_parameter("gw", [P, NI * HIDDEN], bf16, isOutput=False)
    uw = nc.declare_dram_parameter("uw", [P, NI * HIDDEN], bf16, isOutput=False)
    dw = nc.declare_dram_parameter("dw", [NI // 4, P, 4 * HIDDEN], bf16,
                                   isOutput=False)
    cw = nc.declare_dram_parameter("cw", [P, tp], f32, isOutput=False)
    y = nc.declare_dram_parameter("y", [HIDDEN, tp], bf16, isOutput=True)

    H = HIDDEN

    with tile.TileContext(nc) as tc:
        with (
            tc.tile_pool(name="ps", bufs=1, space="PSUM") as ps,
            tc.tile_pool(name="sb", bufs=1) as sb,
            tc.tile_pool(name="sm", bufs=2) as sm,
        ):
            # --- startup-critical DMAs, one per DMA-capable queue ---
            wsrc = sb.tile([P, P], bf16, tag="wsrc", name="wsrc")
            nc.gpsimd.memset(wsrc[:], 0.0)

            # X chunked in k-consumption order. Tiles keep >=2KB per-partition
            # rows — sub-2KB descriptor rows halve early DMA throughput,
            # which is what actually gates the startup.
            xkmap = {}
            xtiles = []
            for ci, ks in enumerate(((0,), (1,), (2, 3), (4, 5), (6, 7))):
                t = sb.tile([P, len(ks) * tp], bf16, tag=f"x{ci}", name=f"x{ci}")
                xtiles.append((t, ks[0], len(ks)))
                for k in ks:
                    xkmap[k] = (t, k - ks[0])
            nc.sync.dma_start(out=xtiles[0][0][:], in_=xt[:, 0:tp])

            def xk(k, c):
                t, off = xkmap[k]
                return t[:, off * tp + c * csz: off * tp + (c + 1) * csz]

            # weight chunk tiles: [start_i, n_i) each; g0/u0 single tiles on
            # their own queues so the first iteration's weights arrive early
            WCH = ((0, 1), (1, 1), (2, 2), (4, 4), (8, 4), (12, 4))
            gts = {}
            uts = {}
            gts[WCH[0]] = sb.tile([P, H], bf16, tag="g0", name="g0")
            nc.scalar.dma_start(out=gts[WCH[0]][:], in_=gw[:, 0:H])
            uts[WCH[0]] = sb.tile([P, H], bf16, tag="u0", name="u0")
            nc.gpsimd.dma_start(out=uts[WCH[0]][:], in_=uw[:, 0:H])

            def wslice(tiles, i, k):
                for (i0, n), t in tiles.items():
                    if i0 <= i < i0 + n:
                        return t[:, (i - i0) * H + k * P:
                                 (i - i0) * H + (k + 1) * P]
                raise KeyError(i)

            # remaining X chunks next on sync, then weights by deadline
            for t, k0, nk in xtiles[1:]:
                nc.sync.dma_start(out=t[:], in_=xt[:, k0 * tp:(k0 + nk) * tp])
            for i0, n in WCH[1:]:
                gts[(i0, n)] = sb.tile([P, n * H], bf16, tag=f"g{i0}",
                                       name=f"g{i0}")
                nc.sync.dma_start(out=gts[(i0, n)][:],
                                  in_=gw[:, i0 * H:(i0 + n) * H])
                uts[(i0, n)] = sb.tile([P, n * H], bf16, tag=f"u{i0}",
                                       name=f"u{i0}")
                nc.sync.dma_start(out=uts[(i0, n)][:],
                                  in_=uw[:, i0 * H:(i0 + n) * H])

            dts = []
            for q in range(4):
                t = sb.tile([P, 4 * H], bf16, tag=f"dw{q}", name=f"dwt{q}")
                nc.sync.dma_start(out=t[:], in_=dw[q])
                dts.append(t)

            def dslice(i, jj):
                q, r = divmod(i, 4)
                return dts[q][:, r * H + jj * P: r * H + (jj + 1) * P]

            cwt = sb.tile([P, ntc, csz], f32, tag="cw", name="cwt")
            nc.sync.dma_start(out=cwt[:], in_=cw[:])

            def pair(idx, name):
                return ps.tile([P, ntc, 512], f32, tag=f"pb{idx}", name=name)

            # --- PE warmup during the input DMA wait ---
            wps = pair(3, "warm_ps")
            for r in range(N_WARMUP):
                nc.tensor.matmul(out=wps[:, ntc - 1, 0:P], lhsT=wsrc[:],
                                 rhs=wsrc[:], start=True, stop=True)

            hts = [sb.tile([P, ntc, csz], bf16, tag=f"h{i}", name=f"ht{i}")
                   for i in range(NI)]

            # ---- Stage A ----
            # chunk c=0 consumed before c=1 per k so the first matmul's DMA
            # dependency is a single half-tile; a few warmup MMs after the
            # first k-groups bridge DMA-arrival jitter without idling the PE
            for i in range(NI):
                pg = pair(2 * (i % 2), f"pg{i}")
                pu = pair(2 * (i % 2) + 1, f"pu{i}")
                for k in range(KH):
                    lg = wslice(gts, i, k)
                    lu = wslice(uts, i, k)
                    for c in range(ntc):
                        nc.tensor.matmul(out=pg[:, c, 0:csz], lhsT=lg,
                                         rhs=xk(k, c),
                                         start=(k == 0), stop=(k == KH - 1))
                    for c in range(ntc):
                        nc.tensor.matmul(out=pu[:, c, 0:csz], lhsT=lu,
                                         rhs=xk(k, c),
                                         start=(k == 0), stop=(k == KH - 1))
                    if i == 0 and k < 2:
                        # bridge X-arrival jitter so HAM never re-throttles
                        for r in range(4):
                            nc.tensor.matmul(out=wps[:, ntc - 1, 0:P],
                                             lhsT=wsrc[:], rhs=wsrc[:],
                                             start=True, stop=True)
                sg = sm.tile([P, ntc, csz], f32, tag="sg", name=f"sg{i}")
                nc.scalar.activation(out=sg[:], in_=pg[:, :, 0:csz],
                                     func=mybir.ActivationFunctionType.Silu)
                nc.vector.tensor_mul(out=hts[i][:], in0=sg[:],
                                     in1=pu[:, :, 0:csz])

            # ---- Stage B ----
            # groups of <=2 jj (<=4 PSUM banks); group 0 overlaps the stage A
            # tail. The final single-jj group's readout is split per chunk
            # with the two DMAs on different queues for the shortest tail.
            jgroups = [(0, 1), (2, 3), (4, 5), (6,), (7,)]
            for g, grp in enumerate(jgroups):
                base = 2 * (g % 2)
                if len(grp) > 1 or ntc == 1:
                    pys = {jj: pair(base + a, f"py{jj}")
                           for a, jj in enumerate(grp)}
                    for i in range(NI):
                        for jj in grp:
                            ld = dslice(i, jj)
                            for c in range(ntc):
                                nc.tensor.matmul(out=pys[jj][:, c, 0:csz],
                                                 lhsT=ld,
                                                 rhs=hts[i][:, c, 0:csz],
                                                 start=(i == 0),
                                                 stop=(i == NI - 1))
                    for a, jj in enumerate(grp):
                        yb = sm.tile([P, ntc, csz], bf16, tag=f"yb{jj % 2}",
                                     name=f"yb{jj}")
                        nc.vector.tensor_mul(out=yb[:],
                                             in0=pys[jj][:, :, 0:csz],
                                             in1=cwt[:])
                        eng = nc.scalar if jj % 2 else nc.sync
                        eng.dma_start(out=y[jj * P:(jj + 1) * P, :], in_=yb[:])
                    continue
                # single-jj tail group: chunk-major so chunk c's readout and
                # DMA hide under chunk c+1's matmuls. Each chunk gets its OWN
                # PSUM tile — sharing the pair tile would make c1's first
                # matmul wait on c0's readout mul (WAR on the tile). The
                # final chunk's DMA is split across both queues.
                jj = grp[0]
                last_grp = g == len(jgroups) - 1
                accs = [pair(base, f"py{jj}c0"), pair(base + 1, f"py{jj}c1")]
                for c in range(ntc):
                    acc = accs[c][:, c, 0:csz]
                    for i in range(NI):
                        nc.tensor.matmul(out=acc,
                                         lhsT=dslice(i, jj),
                                         rhs=hts[i][:, c, 0:csz],
                                         start=(i == 0), stop=(i == NI - 1))
                    ybc = sm.tile([P, csz], bf16, tag=f"ybc{c}",
                                  name=f"ybc{jj}_{c}")
                    nc.vector.tensor_mul(out=ybc[:],
                                         in0=acc,
                                         in1=cwt[:, c, :])
                    if last_grp and c == ntc - 1:
                        half = csz // 2
                        nc.sync.dma_start(
                            out=y[jj * P:(jj + 1) * P, c * csz:c * csz + half],
                            in_=ybc[:, 0:half])
                        nc.scalar.dma_start(
                            out=y[jj * P:(jj + 1) * P,
                                  c * csz + half:(c + 1) * csz],
                            in_=ybc[:, half:csz])
                    else:
                        eng = nc.scalar if c % 2 else nc.sync
                        eng.dma_start(
                            out=y[jj * P:(jj + 1) * P, c * csz:(c + 1) * csz],
                            in_=ybc[:])

    nc.finalize()
    return nc


def _route(expert_indices, expert_weights):
    idx = np.asarray(expert_indices).astype(np.int64)
    wts = np.asarray(expert_weights).astype(np.float32)
    n = idx.shape[0]
    cw_full = np.zeros((N_EXPERTS, n), np.float32)
    for k in range(idx.shape[1]):
        np.add.at(cw_full, (idx[:, k], np.arange(n)), wts[:, k])
    ids = [np.nonzero(cw_full[e])[0] for e in range(N_EXPERTS)]
    maxc = max(len(i) for i in ids)
    t_pad = max(512, ((maxc + 7) // 8) * 8)
    return cw_full, ids, t_pad


def _run(nc, in_maps, trace=False, trace_cores=None):
    from concourse.bass_utils import run_bass_kernel_spmd

    return run_bass_kernel_spmd(
        nc, in_maps, list(range(N_EXPERTS)), trace=trace,
        trace_cores=trace_cores,
    )


def _make_in_maps(tokens, gwl, uwl, dwl, cw_full, ids, t_pad):
    bf16 = np.dtype("bfloat16")
    in_maps = []
    for e in range(N_EXPERTS):
        ce = len(ids[e])
        xe = np.zeros((HIDDEN, t_pad), np.float32)
        xe[:, :ce] = tokens[ids[e]].T
        cwe = np.zeros((t_pad,), np.float32)
        cwe[:ce] = cw_full[e, ids[e]]
        in_maps.append({
            # [P, KH*tp]: xt[p, k*tp+t] = X^T[k*128+p, t]
            "xt": np.ascontiguousarray(
                xe.reshape(KH, P, t_pad).transpose(1, 0, 2)
            ).reshape(P, KH * t_pad).astype(bf16),
            # [P, NI*H]: gw[p, i*H + k*128+q] = G[k*128+p, i*128+q]
            "gw": gwl[e],
            "uw": uwl[e],
            # [4, P, 4*H]: dw[q][p][r*H+h] = D[(4q+r)*128+p, h]
            "dw": dwl[e],
            "cw": np.ascontiguousarray(
                np.broadcast_to(cwe[None, :], (P, t_pad))),
        })
    return in_maps


def prepare(tokens, expert_indices, expert_weights, gate_weight, up_weight,
            down_weight):
    """Host-side routing + layout. Returns (nc, in_maps, ids, t_pad)."""
    tokens = np.ascontiguousarray(np.asarray(tokens, dtype=np.float32))
    gate_weight = np.asarray(gate_weight, dtype=np.float32)
    up_weight = np.asarray(up_weight, dtype=np.float32)
    down_weight = np.asarray(down_weight, dtype=np.float32)

    cw_full, ids, t_pad = _route(expert_indices, expert_weights)

    key = t_pad
    if key not in _CACHE:
        _CACHE[key] = _build(t_pad)
    nc = _CACHE[key]

    bf16 = np.dtype("bfloat16")
    gwl, uwl, dwl = [], [], []
    for e in range(N_EXPERTS):
        gwl.append(np.ascontiguousarray(
            gate_weight[e].reshape(KH, P, NI, P).transpose(1, 2, 0, 3)
        ).reshape(P, NI * HIDDEN).astype(bf16))
        uwl.append(np.ascontiguousarray(
            up_weight[e].reshape(KH, P, NI, P).transpose(1, 2, 0, 3)
        ).reshape(P, NI * HIDDEN).astype(bf16))
        dwl.append(np.ascontiguousarray(
            down_weight[e].reshape(4, 4, P, HIDDEN).transpose(0, 2, 1, 3)
        ).reshape(4, P, 4 * HIDDEN).astype(bf16))

    in_maps = _make_in_maps(tokens, gwl, uwl, dwl, cw_full, ids, t_pad)
    return nc, in_maps, ids, t_pad


def combine(results, ids):
    out = np.zeros((N_TOKENS, HIDDEN), np.float32)
    for e in range(N_EXPERTS):
        ce = len(ids[e])
        out[ids[e]] += results[e]["y"].T[:ce].astype(np.float32)
    return out


def kernel(tokens, expert_indices, expert_weights, gate_weight, up_weight,
           down_weight):
    tokens = np.ascontiguousarray(np.asarray(tokens, dtype=np.float32))
    cw_full, ids, t_pad = _route(expert_indices, expert_weights)
    if t_pad <= 1024:
        nc, in_maps, ids, _ = prepare(tokens, expert_indices, expert_weights,
                                      gate_weight, up_weight, down_weight)
        res = _run(nc, in_maps, trace=False)
        return combine(res.results, ids)

    # fallback for extreme routing skew: split each expert's token list into
    # segments of <=1024 and run one SPMD launch per segment
    gate_weight = np.asarray(gate_weight, dtype=np.float32)
    up_weight = np.asarray(up_weight, dtype=np.float32)
    down_weight = np.asarray(down_weight, dtype=np.float32)
    bf16 = np.dtype("bfloat16")
    gwl, uwl, dwl = [], [], []
    for e in range(N_EXPERTS):
        gwl.append(np.ascontiguousarray(
            gate_weight[e].reshape(KH, P, NI, P).transpose(1, 2, 0, 3)
        ).reshape(P, NI * HIDDEN).astype(bf16))
        uwl.append(np.ascontiguousarray(
            up_weight[e].reshape(KH, P, NI, P).transpose(1, 2, 0, 3)
        ).reshape(P, NI * HIDDEN).astype(bf16))
        dwl.append(np.ascontiguousarray(
            down_weight[e].reshape(4, 4, P, HIDDEN).transpose(0, 2, 1, 3)
        ).reshape(4, P, 4 * HIDDEN).astype(bf16))

    nseg = (max(len(i) for i in ids) + 1023) // 1024
    out = np.zeros((N_TOKENS, HIDDEN), np.float32)
    for s in range(nseg):
        sids = [i[s * 1024:(s + 1) * 1024] for i in ids]
        maxc = max((len(i) for i in sids), default=0)
        if maxc == 0:
            continue
        seg_pad = max(512, ((maxc + 7) // 8) * 8)
        if seg_pad not in _CACHE:
            _CACHE[seg_pad] = _build(seg_pad)
        in_maps = _make_in_maps(tokens, gwl, uwl, dwl, cw_full, sids, seg_pad)
        res = _run(_CACHE[seg_pad], in_maps, trace=False)
        out += combine(res.results, sids)
    return out


# revision 34
# speedup vs baseline: 1.1935x; 1.1892x over previous
"""Expert-parallel MoE FFN kernel for Trainium2 (8 NeuronCores, one expert per core).

Host side: routes tokens to experts (dedup per expert, summing duplicate top-k
weights), pads each expert's token list to the max expert count t_pad (NOT
rounded to 512 — matmul free dim is arbitrary <=512), and pre-tiles the weight
matrices into DMA-friendly contiguous blocks.

Device side (per core, expert e):
  h^T = silu(G_e^T X^T) * (U_e^T X^T)        [I, T]   (stage A)
  y^T = (D^T h^T) * cw                        [H, T]   (stage B)
All matmuls in bf16 (1 col/cycle on the PE), fp32 accumulation in PSUM.

Perf-critical structure (from NTFF trace analysis of previous versions):
 - The token dim is split in two EQUAL chunks (csz = t_pad/2 <= 512); the two
   PSUM accumulators for a (gate|up|out) pair live in ONE [P, 2, 512] tile
   spanning two adjacent banks, so silu / h-mul / cw-mul / y-DMA each cover
   both chunks with a single instruction. Fewer instructions -> fewer
   cross-engine event semaphores -> a much shorter end-of-kernel semaphore
   reset storm (which counts toward measured exec time).
 - Input DMAs are spread across the three DMA-capable queues (sync/scalar/
   gpsimd) with the startup-critical tiles (X k=0, G0, U0) first; X is
   chunked 5 ways so arrival tracks the k-consumption order of the first
   i-iteration. Weight DMAs are batched (fewer events) and ordered by
   deadline on the sync queue.
 - ~38 warmup matmuls on a memset tile run during the initial DMA wait so the
   PE HAM clock-gate reaches 2.4 GHz before real matmuls start (plus a few
   interleaved in the first i-iteration to bridge X-arrival jitter).
 - PSUM is hand-scheduled as 4 explicit pair tiles. Stage A double-buffers
   (even i -> pb0/pb1, odd i -> pb2/pb3); stage B's first accumulator group
   reuses pb0/pb1 (freed mid stage A) so the tensor engine never idles at
   the A->B transition. Stage B's last two groups are single-jj so the
   post-last-matmul tail is one mul + one DMA.
"""
import sys

if "/opt/trn_rl_repo" not in sys.path:
    sys.path.insert(0, "/opt/trn_rl_repo")

import numpy as np

N_TOKENS, TOP_K, N_EXPERTS, HIDDEN, INTER = 4096, 2, 8, 1024, 2048
P = 128
NI = INTER // P          # 16 I-tiles
KH = HIDDEN // P         # 8 H(contraction)-tiles
N_WARMUP = 38

_CACHE = {}


_SEM_PATCHED = False


def _shrink_sem_pool(cap=128):
    """Cap the kernel event-semaphore pool. The NEFF epilogue individually
    resets every semaphore in the pool (~25ns each across the engines); the
    kernel only uses ~20, so the default 254-wide pool costs ~6us of measured
    tail. A smaller pool shrinks that reset storm proportionally."""
    global _SEM_PATCHED
    if _SEM_PATCHED:
        return
    import concourse.bass as bass_mod

    orig = bass_mod.get_kernel_semaphore_range

    def small_range():
        r = orig()
        return range(r.start, min(r.stop, r.start + cap))

    bass_mod.get_kernel_semaphore_range = small_range
    _SEM_PATCHED = True


def _build(t_pad):
    import concourse.bacc as bacc
    import concourse.mybir as mybir
    import concourse.tile as tile

    _shrink_sem_pool()

    f32 = mybir.dt.float32
    bf16 = mybir.dt.bfloat16
    tp = t_pad
    assert tp % 8 == 0 and tp <= 1024, f"t_pad {tp} unsupported"
    ntc = 1 if tp <= 512 else 2
    csz = tp // ntc

    nc = bacc.Bacc()
    xt = nc.declare_dram_parameter("xt", [P, KH * tp], bf16, isOutput=False)
    gw = nc.declare_dram_parameter("gw", [P, NI * HIDDEN], bf16, isOutput=False)
    uw = nc.declare_dram_parameter("uw", [P, NI * HIDDEN], bf16, isOutput=False)
    dw = nc.declare_dram_parameter("dw", [NI // 4, P, 4 * HIDDEN], bf16,
                                   isOutput=False)
    cw = nc.declare_dram_parameter("cw", [P, tp], f32, isOutput=False)
    y = nc.declare_dram_parameter("y", [HIDDEN, tp], bf16, isOutput=True)

    H = HIDDEN

    with tile.TileContext(nc) as tc:
        with (
            tc.tile_pool(name="ps", bufs=1, space="PSUM") as ps,
            tc.tile_pool(name="sb", bufs=1) as sb,
            tc.tile_pool(name="sm", bufs=2) as sm,
        ):
            # --- startup-critical DMAs, one per DMA-capable queue ---
            wsrc = sb.tile([P, P], bf16, tag="wsrc", name="wsrc")
            nc.gpsimd.memset(wsrc[:], 0.0)

            # X chunked in k-consumption order. Tiles keep >=2KB per-partition
            # rows — sub-2KB descriptor rows halve early DMA throughput,
            # which is what actually gates the startup.
            xkmap = {}
            xtiles = []
            for ci, ks in enumerate(((0,), (1,), (2, 3), (4, 5), (6, 7))):
                t = sb.tile([P, len(ks) * tp], bf16, tag=f"x{ci}", name=f"x{ci}")
                xtiles.append((t, ks[0], len(ks)))
                for k in ks:
                    xkmap[k] = (t, k - ks[0])
            nc.sync.dma_start(out=xtiles[0][0][:], in_=xt[:, 0:tp])

            def xk(k, c):
                t, off = xkmap[k]
                return t[:, off * tp + c * csz: off * tp + (c + 1) * csz]

            # weight chunk tiles: [start_i, n_i) each; g0/u0 single tiles on
            # their own queues so the first iteration's weights arrive early
            WCH = ((0, 1), (1, 1), (2, 2), (4, 4), (8, 4), (12, 4))
            gts = {}
            uts = {}
            gts[WCH[0]] = sb.tile([P, H], bf16, tag="g0", name="g0")
            nc.scalar.dma_start(out=gts[WCH[0]][:], in_=gw[:, 0:H])
            uts[WCH[0]] = sb.tile([P, H], bf16, tag="u0", name="u0")
            nc.gpsimd.dma_start(out=uts[WCH[0]][:], in_=uw[:, 0:H])

            def wslice(tiles, i, k):
                for (i0, n), t in tiles.items():
                    if i0 <= i < i0 + n:
                        return t[:, (i - i0) * H + k * P:
                                 (i - i0) * H + (k + 1) * P]
                raise KeyError(i)

            # remaining X chunks next on sync, then weights by deadline
            for t, k0, nk in xtiles[1:]:
                nc.sync.dma_start(out=t[:], in_=xt[:, k0 * tp:(k0 + nk) * tp])
            for i0, n in WCH[1:]:
                gts[(i0, n)] = sb.tile([P, n * H], bf16, tag=f"g{i0}",
                                       name=f"g{i0}")
                nc.sync.dma_start(out=gts[(i0, n)][:],
                                  in_=gw[:, i0 * H:(i0 + n) * H])
                uts[(i0, n)] = sb.tile([P, n * H], bf16, tag=f"u{i0}",
                                       name=f"u{i0}")
                nc.sync.dma_start(out=uts[(i0, n)][:],
                                  in_=uw[:, i0 * H:(i0 + n) * H])

            dts = []
            for q in range(4):
                t = sb.tile([P, 4 * H], bf16, tag=f"dw{q}", name=f"dwt{q}")
                nc.sync.dma_start(out=t[:], in_=dw[q])
                dts.append(t)

            def dslice(i, jj):
                q, r = divmod(i, 4)
                return dts[q][:, r * H + jj * P: r * H + (jj + 1) * P]

            cwt = sb.tile([P, ntc, csz], f32, tag="cw", name="cwt")
            nc.sync.dma_start(out=cwt[:], in_=cw[:])

            def pair(idx, name):
                return ps.tile([P, ntc, 512], f32, tag=f"pb{idx}", name=name)

            # --- PE warmup during the input DMA wait ---
            wps = pair(3, "warm_ps")
            for r in range(N_WARMUP):
                nc.tensor.matmul(out=wps[:, ntc - 1, 0:P], lhsT=wsrc[:],
                                 rhs=wsrc[:], start=True, stop=True)

            hts = [sb.tile([P, ntc, csz], bf16, tag=f"h{i}", name=f"ht{i}")
                   for i in range(NI)]

            # ---- Stage A ----
            # chunk c=0 consumed before c=1 per k so the first matmul's DMA
            # dependency is a single half-tile; a few warmup MMs after the
            # first k-groups bridge DMA-arrival jitter without idling the PE
            for i in range(NI):
                pg = pair(2 * (i % 2), f"pg{i}")
                pu = pair(2 * (i % 2) + 1, f"pu{i}")
                for k in range(KH):
                    lg = wslice(gts, i, k)
                    lu = wslice(uts, i, k)
                    for c in range(ntc):
                        nc.tensor.matmul(out=pg[:, c, 0:csz], lhsT=lg,
                                         rhs=xk(k, c),
                                         start=(k == 0), stop=(k == KH - 1))
                    for c in range(ntc):
                        nc.tensor.matmul(out=pu[:, c, 0:csz], lhsT=lu,
                                         rhs=xk(k, c),
                                         start=(k == 0), stop=(k == KH - 1))
                    if i == 0 and k < 2:
                        # bridge X-arrival jitter so HAM never re-throttles
                        for r in range(4):
                            nc.tensor.matmul(out=wps[:, ntc - 1, 0:P],
                                             lhsT=wsrc[:], rhs=wsrc[:],
                                             start=True, stop=True)
                sg = sm.tile([P, ntc, csz], f32, tag="sg", name=f"sg{i}")
                nc.scalar.activation(out=sg[:], in_=pg[:, :, 0:csz],
                                     func=mybir.ActivationFunctionType.Silu)
                nc.vector.tensor_mul(out=hts[i][:], in0=sg[:],
                                     in1=pu[:, :, 0:csz])

            # ---- Stage B ----
            # groups of <=2 jj (<=4 PSUM banks); group 0 overlaps the stage A
            # tail. The final single-jj group's readout is split per chunk
            # with the two DMAs on different queues for the shortest tail.
            jgroups = [(0, 1), (2, 3), (4, 5), (6,), (7,)]
            for g, grp in enumerate(jgroups):
                base = 2 * (g % 2)
                if len(grp) > 1 or ntc == 1:
                    pys = {jj: pair(base + a, f"py{jj}")
                           for a, jj in enumerate(grp)}
                    for i in range(NI):
                        for jj in grp:
                            ld = dslice(i, jj)
                            for c in range(ntc):
                                nc.tensor.matmul(out=pys[jj][:, c, 0:csz],
                                                 lhsT=ld,
                                                 rhs=hts[i][:, c, 0:csz],
                                                 start=(i == 0),
                                                 stop=(i == NI - 1))
                    for a, jj in enumerate(grp):
                        yb = sm.tile([P, ntc, csz], bf16, tag=f"yb{jj % 2}",
                                     name=f"yb{jj}")
                        nc.vector.tensor_mul(out=yb[:],
                                             in0=pys[jj][:, :, 0:csz],
                                             in1=cwt[:])
                        eng = nc.gpsimd if jj % 2 else nc.sync
                        eng.dma_start(out=y[jj * P:(jj + 1) * P, :], in_=yb[:])
                    continue
                # single-jj tail group: chunk-major so chunk c's readout and
                # DMA hide under chunk c+1's matmuls. Each chunk gets its OWN
                # PSUM tile — sharing the pair tile would make c1's first
                # matmul wait on c0's readout mul (WAR on the tile). The
                # final chunk's DMA is split across both queues.
                jj = grp[0]
                last_grp = g == len(jgroups) - 1
                accs = [pair(base, f"py{jj}c0"), pair(base + 1, f"py{jj}c1")]
                for c in range(ntc):
                    acc = accs[c][:, c, 0:csz]
                    for i in range(NI):
                        nc.tensor.matmul(out=acc,
                                         lhsT=dslice(i, jj),
                                         rhs=hts[i][:, c, 0:csz],
                                         start=(i == 0), stop=(i == NI - 1))
                    ybc = sm.tile([P, csz], bf16, tag=f"ybc{c}",
                                  name=f"ybc{jj}_{c}")
                    nc.vector.tensor_mul(out=ybc[:],
                                         in0=acc,
                                         in1=cwt[:, c, :])
                    if last_grp and c == ntc - 1:
                        half = csz // 2
                        nc.sync.dma_start(
                            out=y[jj * P:(jj + 1) * P, c * csz:c * csz + half],
                            in_=ybc[:, 0:half])
                        nc.gpsimd.dma_start(
                            out=y[jj * P:(jj + 1) * P,
                                  c * csz + half:(c + 1) * csz],
                            in_=ybc[:, half:csz])
                    else:
                        eng = nc.gpsimd if c % 2 else nc.sync
                        eng.dma_start(
                            out=y[jj * P:(jj + 1) * P, c * csz:(c + 1) * csz],
                            in_=ybc[:])

    nc.finalize()
    return nc


def _route(expert_indices, expert_weights):
    idx = np.asarray(expert_indices).astype(np.int64)
    wts = np.asarray(expert_weights).astype(np.float32)
    n = idx.shape[0]
    cw_full = np.zeros((N_EXPERTS, n), np.float32)
    for k in range(idx.shape[1]):
        np.add.at(cw_full, (idx[:, k], np.arange(n)), wts[:, k])
    ids = [np.nonzero(cw_full[e])[0] for e in range(N_EXPERTS)]
    maxc = max(len(i) for i in ids)
    t_pad = max(512, ((maxc + 7) // 8) * 8)
    return cw_full, ids, t_pad


def _run(nc, in_maps, trace=False, trace_cores=None):
    from concourse.bass_utils import run_bass_kernel_spmd

    return run_bass_kernel_spmd(
        nc, in_maps, list(range(N_EXPERTS)), trace=trace,
        trace_cores=trace_cores,
    )


def _make_in_maps(tokens, gwl, uwl, dwl, cw_full, ids, t_pad):
    bf16 = np.dtype("bfloat16")
    in_maps = []
    for e in range(N_EXPERTS):
        ce = len(ids[e])
        xe = np.zeros((HIDDEN, t_pad), np.float32)
        xe[:, :ce] = tokens[ids[e]].T
        cwe = np.zeros((t_pad,), np.float32)
        cwe[:ce] = cw_full[e, ids[e]]
        in_maps.append({
            # [P, KH*tp]: xt[p, k*tp+t] = X^T[k*128+p, t]
            "xt": np.ascontiguousarray(
                xe.reshape(KH, P, t_pad).transpose(1, 0, 2)
            ).reshape(P, KH * t_pad).astype(bf16),
            # [P, NI*H]: gw[p, i*H + k*128+q] = G[k*128+p, i*128+q]
            "gw": gwl[e],
            "uw": uwl[e],
            # [4, P, 4*H]: dw[q][p][r*H+h] = D[(4q+r)*128+p, h]
            "dw": dwl[e],
            "cw": np.ascontiguousarray(
                np.broadcast_to(cwe[None, :], (P, t_pad))),
        })
    return in_maps


def prepare(tokens, expert_indices, expert_weights, gate_weight, up_weight,
            down_weight):
    """Host-side routing + layout. Returns (nc, in_maps, ids, t_pad)."""
    tokens = np.ascontiguousarray(np.asarray(tokens, dtype=np.float32))
    gate_weight = np.asarray(gate_weight, dtype=np.float32)
    up_weight = np.asarray(up_weight, dtype=np.float32)
    down_weight = np.asarray(down_weight, dtype=np.float32)

    cw_full, ids, t_pad = _route(expert_indices, expert_weights)

    key = t_pad
    if key not in _CACHE:
        _CACHE[key] = _build(t_pad)
    nc = _CACHE[key]

    bf16 = np.dtype("bfloat16")
    gwl, uwl, dwl = [], [], []
    for e in range(N_EXPERTS):
        gwl.append(np.ascontiguousarray(
            gate_weight[e].reshape(KH, P, NI, P).transpose(1, 2, 0, 3)
        ).reshape(P, NI * HIDDEN).astype(bf16))
        uwl.append(np.ascontiguousarray(
            up_weight[e].reshape(KH, P, NI, P).transpose(1, 2, 0, 3)
        ).reshape(P, NI * HIDDEN).astype(bf16))
        dwl.append(np.ascontiguousarray(
            down_weight[e].reshape(4, 4, P, HIDDEN).transpose(0, 2, 1, 3)
        ).reshape(4, P, 4 * HIDDEN).astype(bf16))

    in_maps = _make_in_maps(tokens, gwl, uwl, dwl, cw_full, ids, t_pad)
    return nc, in_maps, ids, t_pad


def combine(results, ids):
    out = np.zeros((N_TOKENS, HIDDEN), np.float32)
    for e in range(N_EXPERTS):
        ce = len(ids[e])
        out[ids[e]] += results[e]["y"].T[:ce].astype(np.float32)
    return out


def kernel(tokens, expert_indices, expert_weights, gate_weight, up_weight,
           down_weight):
    tokens = np.ascontiguousarray(np.asarray(tokens, dtype=np.float32))
    cw_full, ids, t_pad = _route(expert_indices, expert_weights)
    if t_pad <= 1024:
        nc, in_maps, ids, _ = prepare(tokens, expert_indices, expert_weights,
                                      gate_weight, up_weight, down_weight)
        res = _run(nc, in_maps, trace=False)
        return combine(res.results, ids)

    # fallback for extreme routing skew: split each expert's token list into
    # segments of <=1024 and run one SPMD launch per segment
    gate_weight = np.asarray(gate_weight, dtype=np.float32)
    up_weight = np.asarray(up_weight, dtype=np.float32)
    down_weight = np.asarray(down_weight, dtype=np.float32)
    bf16 = np.dtype("bfloat16")
    gwl, uwl, dwl = [], [], []
    for e in range(N_EXPERTS):
        gwl.append(np.ascontiguousarray(
            gate_weight[e].reshape(KH, P, NI, P).transpose(1, 2, 0, 3)
        ).reshape(P, NI * HIDDEN).astype(bf16))
        uwl.append(np.ascontiguousarray(
            up_weight[e].reshape(KH, P, NI, P).transpose(1, 2, 0, 3)
        ).reshape(P, NI * HIDDEN).astype(bf16))
        dwl.append(np.ascontiguousarray(
            down_weight[e].reshape(4, 4, P, HIDDEN).transpose(0, 2, 1, 3)
        ).reshape(4, P, 4 * HIDDEN).astype(bf16))

    nseg = (max(len(i) for i in ids) + 1023) // 1024
    out = np.zeros((N_TOKENS, HIDDEN), np.float32)
    for s in range(nseg):
        sids = [i[s * 1024:(s + 1) * 1024] for i in ids]
        maxc = max((len(i) for i in sids), default=0)
        if maxc == 0:
            continue
        seg_pad = max(512, ((maxc + 7) // 8) * 8)
        if seg_pad not in _CACHE:
            _CACHE[seg_pad] = _build(seg_pad)
        in_maps = _make_in_maps(tokens, gwl, uwl, dwl, cw_full, sids, seg_pad)
        res = _run(_CACHE[seg_pad], in_maps, trace=False)
        out += combine(res.results, sids)
    return out
